# revision 35
# baseline (speedup 1.0000x reference)
"""Additive (Bahdanau) attention scores on 8 Trainium2 NeuronCores.

scores[b,h,q,k] = sum_d V_w[d] * tanh(q_proj[b,h,q,d] + k_proj[b,h,k,d]) + V_b

Sharding: B*H = 16 heads split across 8 cores (2 heads/core), no comms.

Per-core pipeline (per head, Q=K=512, D=64):
  - query/keys loaded with fast contiguous DMA, transposed on TensorE
    (4x [128,64] -> PSUM [64,128] against an identity), staged to SBUF with
    a ones-row appended.
  - qpT[e,q] / kpT[e,k] projections via PE matmul (bias folded in through the
    ones-row, contraction = 65).  Written into both PSUM partition halves:
    QB holds (qpT[:,c], qpT[:,c+1]) stacked so column 2j is the per-partition
    bias vector for query pair (2j, 2j+1); KD holds kpT duplicated.
  - VectorE tensor_scalar_add: sums[(pair-slot,d), k] = KD + QB[:, 2j] (bf16).
  - ScalarE tanh over 16-pair batches (free dim 8192) - the bottleneck
    (~33.5M activations/core ~ 224us at 1 elem/lane/cycle @ 1.2 GHz).
  - TensorE reduce over d: per pair one matmul with a sparse V_w stationary
    accumulating into a 32-partition column group of a PSUM bank
    (16 pairs per group -> [128, 512] bank = 128 q x 512 k scores).
  - VectorE eviction PSUM->SBUF adds V_b (delayed two blocks so the in-order
    DVE queue never stalls the tanh pipeline on PE), DMA to DRAM.
"""

import sys

if "/opt/trn_rl_repo" not in sys.path:
    sys.path.insert(0, "/opt/trn_rl_repo")

import numpy as np

B, H, Q, K, D = 2, 8, 512, 512, 64
N_CORES = 8
HEADS_PER_CORE = (B * H) // N_CORES  # 2

_BUILT = {}


def _build_nc():
    import os

    import concourse.bacc as bacc
    import concourse.tile as tile
    import concourse.mybir as mybir

    f32 = mybir.dt.float32
    bf16 = mybir.dt.bfloat16
    Tanh = mybir.ActivationFunctionType.Tanh

    nc = bacc.Bacc("TRN2", target_bir_lowering=False, debug=False,
                   num_devices=N_CORES)

    q_d = nc.declare_dram_parameter("query", [HEADS_PER_CORE, Q, D], f32, isOutput=False)
    k_d = nc.declare_dram_parameter("keys", [HEADS_PER_CORE, K, D], f32, isOutput=False)
    waw_d = nc.declare_dram_parameter("Wa_w", [D, D], f32, isOutput=False)
    wab_d = nc.declare_dram_parameter("Wa_b", [D], f32, isOutput=False)
    uaw_d = nc.declare_dram_parameter("Ua_w", [D, D], f32, isOutput=False)
    uab_d = nc.declare_dram_parameter("Ua_b", [D], f32, isOutput=False)
    vw_d = nc.declare_dram_parameter("V_w", [D], f32, isOutput=False)
    vb_d = nc.declare_dram_parameter("V_b", [1], f32, isOutput=False)
    out_d = nc.declare_dram_parameter("out", [HEADS_PER_CORE, Q, K], f32, isOutput=True)

    PAIRS = Q // 2              # 256 query pairs per head
    # Block schedule: pairs per tanh batch.  Big blocks amortize the
    # cross-engine handoff (~2.3us each); small blocks at the very start
    # (short first-ACT latency) and very end (short drain tail).
    if os.environ.get("K_BLK"):
        BLK = int(os.environ["K_BLK"])
        sched_first = [BLK] * (PAIRS // BLK)
        sched_last = sched_first
    else:
        sched_first = [16, 16, 32, 64, 64, 64]
        sched_last = [64, 64, 64, 32, 16, 16]
    NBLK = int(os.environ.get("K_NBLK", "0"))
    SBUFS = int(os.environ.get("K_SBUFS", "2"))
    N_HEADS = int(os.environ.get("K_HEADS", HEADS_PER_CORE))
    INPLACE = bool(int(os.environ.get("K_INPLACE", "1")))

    with tile.TileContext(nc) as tc:
        with (
            tc.tile_pool(name="const", bufs=1) as cpool,
            tc.tile_pool(name="inp", bufs=2) as ipool,
            tc.tile_pool(name="proj_in", bufs=2) as ppool,
            tc.tile_pool(name="proj_out", bufs=2) as opool,
            tc.tile_pool(name="sums", bufs=SBUFS) as spool,
            tc.tile_pool(name="tanh", bufs=SBUFS) as tpool,
            tc.tile_pool(name="stage", bufs=3) as gpool,
            tc.tile_pool(name="ps_proj", bufs=1, space="PSUM") as ps_proj,
            tc.tile_pool(name="ps_score", bufs=4, space="PSUM") as ps_score,
            tc.tile_pool(name="ps_misc", bufs=1, space="PSUM") as ps_misc,
        ):
            # ---- constants ----
            # identity for PE transposes
            ident = cpool.tile([128, 128], f32, tag="ident")
            nc.vector.memset(ident[:], 1.0)
            nc.gpsimd.affine_select(
                ident[:], ident[:], pattern=[[-1, 128]],
                compare_op=mybir.AluOpType.is_equal, fill=0.0,
                base=0, channel_multiplier=1)

            # Wa^T / Ua^T with bias row stacked below: [65, 64]
            waS = cpool.tile([D + 1, D], f32, tag="waS")
            nc.sync.dma_start(waS[0:D, :], waw_d.ap().rearrange("e d -> d e"))
            nc.sync.dma_start(waS[D:D + 1, :], wab_d.ap()[None, :])
            uaS = cpool.tile([D + 1, D], f32, tag="uaS")
            nc.sync.dma_start(uaS[0:D, :], uaw_d.ap().rearrange("e d -> d e"))
            nc.sync.dma_start(uaS[D:D + 1, :], uab_d.ap()[None, :])

            # V_w replicated on both partition halves -> bf16
            vw_rep = cpool.tile([128, 1], f32, tag="vw_rep")
            nc.sync.dma_start(vw_rep[0:D, :], vw_d.ap()[:, None])
            nc.sync.dma_start(vw_rep[D:2 * D, :], vw_d.ap()[:, None])
            vw_bf = cpool.tile([128, 1], bf16, tag="vw_bf")
            nc.vector.tensor_copy(vw_bf[:], vw_rep[:])

            # Sparse stationary bank: block s (cols 32s..32s+31) has V_w at
            # local cols 2s (top half) and 2s+1 (bottom half), zero elsewhere.
            vwall = cpool.tile([128, 512], bf16, tag="vwall")
            nc.vector.memset(vwall[:], 0.0)
            for s in range(16):
                c = 34 * s
                nc.vector.tensor_copy(vwall[0:D, c:c + 1], vw_bf[0:D, :])
                nc.vector.tensor_copy(vwall[D:128, c + 1:c + 2], vw_bf[D:128, :])

            # V_b broadcast to [128, 1] via a rank-1 PE matmul with ones.
            ones_row = cpool.tile([1, 128], f32, tag="ones_row")
            nc.vector.memset(ones_row[:], 1.0)
            vb_sb = cpool.tile([1, 1], f32, tag="vb_sb")
            nc.sync.dma_start(vb_sb[:], vb_d.ap()[None, :])
            vb_ps = ps_misc.tile([128, 1], f32, tag="vb_ps")
            nc.tensor.matmul(vb_ps[:], ones_row[:], vb_sb[:], start=True, stop=True)
            vb_rep = cpool.tile([128, 1], f32, tag="vb_rep")
            nc.vector.tensor_copy(vb_rep[:], vb_ps[:])

            def load_transposed(src_ap, dst, tag):
                """src [512, 64] DRAM -> dst[0:64, 0:512] SBUF (= src.T),
                via contiguous DMA + 4 PE transposes."""
                nt = ipool.tile([128, 4 * D], f32, tag=f"nt_{tag}")
                nc.sync.dma_start(
                    nt[:].rearrange("p (t d) -> p t d", t=4),
                    src_ap.rearrange("(t p) d -> p t d", p=128))
                tp_ps = ps_proj.tile([D, 4 * 128], f32, tag="tp_ps")
                for t in range(4):
                    nc.tensor.transpose(
                        tp_ps[:, 128 * t:128 * (t + 1)],
                        nt[:, D * t:D * (t + 1)], ident[:])
                nc.vector.tensor_copy(dst[0:D, :], tp_ps[:])
                nc.vector.memset(dst[D:D + 1, :], 1.0)

            # deferred eviction queue: (psum_tile, head, bank)
            pending = []

            def flush_evict():
                while pending:
                    ps_tile, eh, ebank = pending.pop(0)
                    stage = gpool.tile([128, K], f32, tag="stage")
                    nc.vector.tensor_scalar_add(stage[:], ps_tile[:], vb_rep[:])
                    nc.sync.dma_start(
                        out_d[eh, 128 * ebank:128 * (ebank + 1), :], stage[:])

            def full_body():
              for h in range(N_HEADS):
                # ---- projections ----
                qS = ppool.tile([D + 1, Q], f32, tag="qS")
                load_transposed(q_d[h], qS, "q")
                kS = ppool.tile([D + 1, K], f32, tag="kS")
                load_transposed(k_d[h], kS, "k")

                # qpT[e, q] on top half; shifted by one q column on bottom half
                qp_ps = ps_proj.tile([128, Q], f32, tag="qp_ps")
                nc.tensor.matmul(qp_ps[0:D, 0:Q], waS[:], qS[:], start=True, stop=True)
                nc.tensor.matmul(qp_ps[D:2 * D, 0:Q - 1], waS[:], qS[:, 1:Q],
                                 start=True, stop=True)
                qb = opool.tile([128, Q], f32, tag="qb")
                nc.vector.tensor_copy(qb[0:D, 0:Q], qp_ps[0:D, 0:Q])
                nc.vector.tensor_copy(qb[D:128, 0:Q - 1], qp_ps[D:128, 0:Q - 1])

                # kpT[e, k] duplicated on both halves
                kp_ps = ps_proj.tile([128, K], f32, tag="kp_ps")
                nc.tensor.matmul(kp_ps[0:D, :], uaS[:], kS[:], start=True, stop=True)
                nc.tensor.matmul(kp_ps[D:2 * D, :], uaS[:], kS[:], start=True, stop=True)
                kd = opool.tile([128, K], bf16, tag="kd")
                nc.vector.tensor_copy(kd[:], kp_ps[:])

                # ---- main loop ----
                sched = sched_last if h == N_HEADS - 1 else sched_first
                if NBLK:  # reduced-size debug runs
                    sched = sched[:NBLK]
                score_ps = None
                j0 = 0
                for blk, blksz in enumerate(sched):
                    fd = blksz * K
                    sums = spool.tile([128, fd], bf16, tag="sums")
                    for i in range(blksz):
                        j = j0 + i
                        nc.vector.tensor_scalar_add(
                            sums[:, i * K:(i + 1) * K], kd[:],
                            qb[:, 2 * j:2 * j + 1])

                    flush_evict()

                    if INPLACE:
                        th = sums
                        nc.scalar.activation(th[:], sums[:], Tanh)
                    else:
                        th = tpool.tile([128, fd], bf16, tag="th")
                        nc.scalar.activation(th[:], sums[:], Tanh)

                    for i in range(blksz):
                        j = j0 + i
                        jb = j % 64      # pair within the PSUM bank
                        c = jb // 16     # 32-partition column group
                        s = jb % 16      # slot within the group
                        if jb == 0:
                            score_ps = ps_score.tile([128, K], f32, tag="score_ps")
                        nc.tensor.matmul(
                            score_ps[32 * c:32 * (c + 1), :],
                            vwall[:, 32 * s:32 * (s + 1)],
                            th[:, i * K:(i + 1) * K],
                            start=(s == 0), stop=(s == 15),
                            tile_position=(0, 32 * c))
                        if jb == 63:
                            pending.append((score_ps, h, j // 64))
                    j0 += blksz

              flush_evict()

            LOOP_R = int(os.environ.get("K_LOOP", "0"))
            if LOOP_R > 1:
                with tc.For_i(0, LOOP_R, 1, hint_engines=(
                        mybir.EngineType.PE, mybir.EngineType.DVE,
                        mybir.EngineType.Activation, mybir.EngineType.SP,
                        mybir.EngineType.Pool)):
                    full_body()
            else:
                full_body()

    nc.compile()
    return nc


def _get_nc():
    if "nc" not in _BUILT:
        _BUILT["nc"] = _build_nc()
    return _BUILT["nc"]


def _shard_inputs(inputs):
    q = np.asarray(inputs["query"], dtype=np.float32).reshape(B * H, Q, D)
    k = np.asarray(inputs["keys"], dtype=np.float32).reshape(B * H, K, D)
    full = {
        "Wa_w": np.ascontiguousarray(np.asarray(inputs["Wa_w"], dtype=np.float32)),
        "Wa_b": np.ascontiguousarray(np.asarray(inputs["Wa_b"], dtype=np.float32)),
        "Ua_w": np.ascontiguousarray(np.asarray(inputs["Ua_w"], dtype=np.float32)),
        "Ua_b": np.ascontiguousarray(np.asarray(inputs["Ua_b"], dtype=np.float32)),
        "V_w": np.ascontiguousarray(np.asarray(inputs["V_w"], dtype=np.float32)),
        "V_b": np.ascontiguousarray(np.asarray(inputs["V_b"], dtype=np.float32)),
    }
    in_maps = []
    for i in range(N_CORES):
        m = dict(full)
        m["query"] = np.ascontiguousarray(q[HEADS_PER_CORE * i:HEADS_PER_CORE * (i + 1)])
        m["keys"] = np.ascontiguousarray(k[HEADS_PER_CORE * i:HEADS_PER_CORE * (i + 1)])
        in_maps.append(m)
    return in_maps


def _run(inputs, trace=False):
    import time

    from concourse.bass_utils import run_bass_kernel_spmd

    nc = _get_nc()
    in_maps = _shard_inputs(inputs)
    res = None
    last_exc = None
    for attempt in range(4):
        try:
            t0 = time.perf_counter()
            res = run_bass_kernel_spmd(nc, in_maps, core_ids=list(range(N_CORES)),
                                       trace=trace)
            res.wall_s = time.perf_counter() - t0
            break
        except Exception as e:  # flaky NRT_EXEC_UNIT_UNRECOVERABLE on axon
            last_exc = e
            try:  # drop the poisoned PJRT client so the retry gets a fresh one
                import jax

                jax.clear_backends()
            except Exception:
                pass
            time.sleep(2.0)
    if res is None:
        raise last_exc
    parts = [np.asarray(res.results[i]["out"]) for i in range(N_CORES)]
    out = np.concatenate(parts, axis=0).reshape(B, H, Q, K).astype(np.float32)
    return out, res


def kernel(**inputs) -> np.ndarray:
    out, _ = _run(inputs, trace=False)
    return out


# revision 39
# speedup vs baseline: 1.2069x; 1.2069x over previous
"""Additive (Bahdanau) attention scores on 8 Trainium2 NeuronCores.

scores[b,h,q,k] = sum_d V_w[d] * tanh(q_proj[b,h,q,d] + k_proj[b,h,k,d]) + V_b

Sharding: B*H = 16 heads split across 8 cores (2 heads/core), no comms.

Per-core pipeline (per head, Q=K=512, D=64):
  - query/keys loaded with fast contiguous DMA, transposed on TensorE
    (4x [128,64] -> PSUM [64,128] against an identity), staged to SBUF with
    a ones-row appended.
  - qpT[e,q] / kpT[e,k] projections via PE matmul (bias folded in through the
    ones-row, contraction = 65).  Written into both PSUM partition halves:
    QB holds (qpT[:,c], qpT[:,c+1]) stacked so column 2j is the per-partition
    bias vector for query pair (2j, 2j+1); KD holds kpT duplicated.
  - VectorE tensor_scalar_add: sums[(pair-slot,d), k] = KD + QB[:, 2j] (bf16).
  - ScalarE tanh over 16-pair batches (free dim 8192) - the bottleneck
    (~33.5M activations/core ~ 224us at 1 elem/lane/cycle @ 1.2 GHz).
  - TensorE reduce over d: per pair one matmul with a sparse V_w stationary
    accumulating into a 32-partition column group of a PSUM bank
    (16 pairs per group -> [128, 512] bank = 128 q x 512 k scores).
  - VectorE eviction PSUM->SBUF adds V_b (delayed two blocks so the in-order
    DVE queue never stalls the tanh pipeline on PE), DMA to DRAM.
"""

import sys

if "/opt/trn_rl_repo" not in sys.path:
    sys.path.insert(0, "/opt/trn_rl_repo")

import numpy as np

B, H, Q, K, D = 2, 8, 512, 512, 64
N_CORES = 8
HEADS_PER_CORE = (B * H) // N_CORES  # 2

_BUILT = {}


def _build_nc():
    import os

    import concourse.bacc as bacc
    import concourse.tile as tile
    import concourse.mybir as mybir

    f32 = mybir.dt.float32
    bf16 = mybir.dt.bfloat16
    Tanh = mybir.ActivationFunctionType.Tanh

    nc = bacc.Bacc("TRN2", target_bir_lowering=False, debug=False,
                   num_devices=N_CORES)

    q_d = nc.declare_dram_parameter("query", [HEADS_PER_CORE, Q, D], f32, isOutput=False)
    k_d = nc.declare_dram_parameter("keys", [HEADS_PER_CORE, K, D], f32, isOutput=False)
    waw_d = nc.declare_dram_parameter("Wa_w", [D, D], f32, isOutput=False)
    wab_d = nc.declare_dram_parameter("Wa_b", [D], f32, isOutput=False)
    uaw_d = nc.declare_dram_parameter("Ua_w", [D, D], f32, isOutput=False)
    uab_d = nc.declare_dram_parameter("Ua_b", [D], f32, isOutput=False)
    vw_d = nc.declare_dram_parameter("V_w", [D], f32, isOutput=False)
    vb_d = nc.declare_dram_parameter("V_b", [1], f32, isOutput=False)
    out_d = nc.declare_dram_parameter("out", [HEADS_PER_CORE, Q, K], f32, isOutput=True)

    PAIRS = Q // 2              # 256 query pairs per head
    # Per-head block schedules (pairs per tanh batch), interleaved so the
    # two heads form independent pipelines: while one chain waits on its
    # DVE->ACT handoff, ACT runs the other chain's ready block.  Small
    # blocks at the global start (short first-ACT latency) and end (short
    # drain tail).  Every block is a multiple of 16 (one PSUM col group).
    schedA = [16, 16] + [32] * 7
    schedB = [32] * 7 + [16, 16]
    assert sum(schedA) == PAIRS and sum(schedB) == PAIRS
    SBUFS = int(os.environ.get("K_SBUFS", "2"))

    with tile.TileContext(nc) as tc:
        with (
            tc.tile_pool(name="const", bufs=1) as cpool,
            tc.tile_pool(name="inp", bufs=2) as ipool,
            tc.tile_pool(name="proj_in", bufs=2) as ppool,
            tc.tile_pool(name="proj_out", bufs=2) as opool,
            tc.tile_pool(name="sums", bufs=SBUFS) as spool,
            tc.tile_pool(name="stage", bufs=3) as gpool,
            tc.tile_pool(name="ps_proj", bufs=1, space="PSUM") as ps_proj,
            tc.tile_pool(name="ps_score", bufs=4, space="PSUM") as ps_score,
            tc.tile_pool(name="ps_misc", bufs=1, space="PSUM") as ps_misc,
        ):
            # ---- constants ----
            # identity for PE transposes
            ident = cpool.tile([128, 128], f32, tag="ident")
            nc.vector.memset(ident[:], 1.0)
            nc.gpsimd.affine_select(
                ident[:], ident[:], pattern=[[-1, 128]],
                compare_op=mybir.AluOpType.is_equal, fill=0.0,
                base=0, channel_multiplier=1)

            # Wa^T / Ua^T with bias row stacked below: [65, 64]
            waS = cpool.tile([D + 1, D], f32, tag="waS")
            nc.sync.dma_start(waS[0:D, :], waw_d.ap().rearrange("e d -> d e"))
            nc.sync.dma_start(waS[D:D + 1, :], wab_d.ap()[None, :])
            uaS = cpool.tile([D + 1, D], f32, tag="uaS")
            nc.sync.dma_start(uaS[0:D, :], uaw_d.ap().rearrange("e d -> d e"))
            nc.sync.dma_start(uaS[D:D + 1, :], uab_d.ap()[None, :])

            # V_w replicated on both partition halves -> bf16
            vw_rep = cpool.tile([128, 1], f32, tag="vw_rep")
            nc.sync.dma_start(vw_rep[0:D, :], vw_d.ap()[:, None])
            nc.sync.dma_start(vw_rep[D:2 * D, :], vw_d.ap()[:, None])
            vw_bf = cpool.tile([128, 1], bf16, tag="vw_bf")
            nc.vector.tensor_copy(vw_bf[:], vw_rep[:])

            # Sparse stationary bank: block s (cols 32s..32s+31) has V_w at
            # local cols 2s (top half) and 2s+1 (bottom half), zero elsewhere.
            vwall = cpool.tile([128, 512], bf16, tag="vwall")
            nc.vector.memset(vwall[:], 0.0)
            for s in range(16):
                c = 34 * s
                nc.vector.tensor_copy(vwall[0:D, c:c + 1], vw_bf[0:D, :])
                nc.vector.tensor_copy(vwall[D:128, c + 1:c + 2], vw_bf[D:128, :])

            # V_b broadcast to [128, 1] via a rank-1 PE matmul with ones.
            ones_row = cpool.tile([1, 128], f32, tag="ones_row")
            nc.vector.memset(ones_row[:], 1.0)
            vb_sb = cpool.tile([1, 1], f32, tag="vb_sb")
            nc.sync.dma_start(vb_sb[:], vb_d.ap()[None, :])
            vb_ps = ps_misc.tile([128, 1], f32, tag="vb_ps")
            nc.tensor.matmul(vb_ps[:], ones_row[:], vb_sb[:], start=True, stop=True)
            vb_rep = cpool.tile([128, 1], f32, tag="vb_rep")
            nc.vector.tensor_copy(vb_rep[:], vb_ps[:])

            def load_transposed(src_ap, dst, tag):
                """src [512, 64] DRAM -> dst[0:64, 0:512] SBUF (= src.T),
                via contiguous DMA + 4 PE transposes."""
                nt = ipool.tile([128, 4 * D], f32, tag=f"nt_{tag}")
                nc.sync.dma_start(
                    nt[:].rearrange("p (t d) -> p t d", t=4),
                    src_ap.rearrange("(t p) d -> p t d", p=128))
                tp_ps = ps_proj.tile([D, 4 * 128], f32, tag="tp_ps")
                for t in range(4):
                    nc.tensor.transpose(
                        tp_ps[:, 128 * t:128 * (t + 1)],
                        nt[:, D * t:D * (t + 1)], ident[:])
                nc.vector.tensor_copy(dst[0:D, :], tp_ps[:])
                nc.vector.memset(dst[D:D + 1, :], 1.0)

            # deferred eviction queue: (psum_tile, head, bank)
            pending = []

            def flush_evict():
                while pending:
                    ps_tile, eh, ebank = pending.pop(0)
                    stage = gpool.tile([128, K], f32, tag="stage")
                    nc.vector.tensor_scalar_add(stage[:], ps_tile[:], vb_rep[:])
                    nc.sync.dma_start(
                        out_d[eh, 128 * ebank:128 * (ebank + 1), :], stage[:])

            def project(h):
                """Load + project head h -> per-head (qb, kd) SBUF tiles."""
                qS = ppool.tile([D + 1, Q], f32, tag="qS")
                load_transposed(q_d[h], qS, "q")
                kS = ppool.tile([D + 1, K], f32, tag="kS")
                load_transposed(k_d[h], kS, "k")

                # qpT[e, q] on top half; shifted by one q column on bottom
                qp_ps = ps_proj.tile([128, Q], f32, tag="qp_ps")
                nc.tensor.matmul(qp_ps[0:D, 0:Q], waS[:], qS[:],
                                 start=True, stop=True)
                nc.tensor.matmul(qp_ps[D:2 * D, 0:Q - 1], waS[:], qS[:, 1:Q],
                                 start=True, stop=True)
                qb = opool.tile([128, Q], f32, tag="qb")
                nc.vector.tensor_copy(qb[0:D, 0:Q], qp_ps[0:D, 0:Q])
                nc.vector.tensor_copy(qb[D:128, 0:Q - 1], qp_ps[D:128, 0:Q - 1])

                # kpT[e, k] duplicated on both halves
                kp_ps = ps_proj.tile([128, K], f32, tag="kp_ps")
                nc.tensor.matmul(kp_ps[0:D, :], uaS[:], kS[:],
                                 start=True, stop=True)
                nc.tensor.matmul(kp_ps[D:2 * D, :], uaS[:], kS[:],
                                 start=True, stop=True)
                kd = opool.tile([128, K], bf16, tag="kd")
                nc.vector.tensor_copy(kd[:], kp_ps[:])
                return {"h": h, "qb": qb, "kd": kd, "j0": 0, "score_ps": None}

            def emit_block(ch, blksz):
                h, qb, kd, j0 = ch["h"], ch["qb"], ch["kd"], ch["j0"]
                fd = blksz * K
                sums = spool.tile([128, fd], bf16, tag=f"sums{h}")
                for i in range(blksz):
                    j = j0 + i
                    nc.vector.tensor_scalar_add(
                        sums[:, i * K:(i + 1) * K], kd[:],
                        qb[:, 2 * j:2 * j + 1])

                flush_evict()

                nc.scalar.activation(sums[:], sums[:], Tanh)
                th = sums

                for i in range(blksz):
                    j = j0 + i
                    jb = j % 64      # pair within the PSUM bank
                    c = jb // 16     # 32-partition column group
                    s = jb % 16      # slot within the group
                    if jb == 0:
                        score_ps = ps_score.tile([128, K], f32, tag="score_ps")
                        ch["score_ps"] = score_ps
                    nc.tensor.matmul(
                        ch["score_ps"][32 * c:32 * (c + 1), :],
                        vwall[:, 32 * s:32 * (s + 1)],
                        th[:, i * K:(i + 1) * K],
                        start=(s == 0), stop=(s == 15),
                        tile_position=(0, 32 * c))
                    if jb == 63:
                        pending.append((ch["score_ps"], h, j // 64))
                ch["j0"] = j0 + blksz

            def full_body():
                chA = project(0)
                emit_block(chA, schedA[0])
                emit_block(chA, schedA[1])
                chB = project(1)
                # steady state: alternate chains, chain B two blocks behind
                for t in range(2, len(schedA)):
                    emit_block(chA, schedA[t])
                    emit_block(chB, schedB[t - 2])
                for t in range(len(schedA) - 2, len(schedB)):
                    emit_block(chB, schedB[t])
                flush_evict()

            LOOP_R = int(os.environ.get("K_LOOP", "0"))
            if LOOP_R > 1:
                with tc.For_i(0, LOOP_R, 1, hint_engines=(
                        mybir.EngineType.PE, mybir.EngineType.DVE,
                        mybir.EngineType.Activation, mybir.EngineType.SP,
                        mybir.EngineType.Pool)):
                    full_body()
            else:
                full_body()

    nc.compile()
    return nc


def _get_nc():
    if "nc" not in _BUILT:
        _BUILT["nc"] = _build_nc()
    return _BUILT["nc"]


def _shard_inputs(inputs):
    q = np.asarray(inputs["query"], dtype=np.float32).reshape(B * H, Q, D)
    k = np.asarray(inputs["keys"], dtype=np.float32).reshape(B * H, K, D)
    full = {
        "Wa_w": np.ascontiguousarray(np.asarray(inputs["Wa_w"], dtype=np.float32)),
        "Wa_b": np.ascontiguousarray(np.asarray(inputs["Wa_b"], dtype=np.float32)),
        "Ua_w": np.ascontiguousarray(np.asarray(inputs["Ua_w"], dtype=np.float32)),
        "Ua_b": np.ascontiguousarray(np.asarray(inputs["Ua_b"], dtype=np.float32)),
        "V_w": np.ascontiguousarray(np.asarray(inputs["V_w"], dtype=np.float32)),
        "V_b": np.ascontiguousarray(np.asarray(inputs["V_b"], dtype=np.float32)),
    }
    in_maps = []
    for i in range(N_CORES):
        m = dict(full)
        m["query"] = np.ascontiguousarray(q[HEADS_PER_CORE * i:HEADS_PER_CORE * (i + 1)])
        m["keys"] = np.ascontiguousarray(k[HEADS_PER_CORE * i:HEADS_PER_CORE * (i + 1)])
        in_maps.append(m)
    return in_maps


def _run(inputs, trace=False):
    import time

    from concourse.bass_utils import run_bass_kernel_spmd

    nc = _get_nc()
    in_maps = _shard_inputs(inputs)
    res = None
    last_exc = None
    for attempt in range(4):
        try:
            t0 = time.perf_counter()
            res = run_bass_kernel_spmd(nc, in_maps, core_ids=list(range(N_CORES)),
                                       trace=trace)
            res.wall_s = time.perf_counter() - t0
            break
        except Exception as e:  # flaky NRT_EXEC_UNIT_UNRECOVERABLE on axon
            last_exc = e
            try:  # drop the poisoned PJRT client so the retry gets a fresh one
                import jax

                jax.clear_backends()
            except Exception:
                pass
            time.sleep(2.0)
    if res is None:
        raise last_exc
    parts = [np.asarray(res.results[i]["out"]) for i in range(N_CORES)]
    out = np.concatenate(parts, axis=0).reshape(B, H, Q, K).astype(np.float32)
    return out, res


def kernel(**inputs) -> np.ndarray:
    out, _ = _run(inputs, trace=False)
    return out
